# revision 1
# baseline (speedup 1.0000x reference)
"""Trainium2 Bass kernel for nn_CSABlock (dual spatial-attention gating).

Reference computation:
    sa_x  = sigmoid(conv3d(concat[max_c(x), mean_c(x)], w, k=7, pad=3))
    fix_out  = move * sa_fix + fix
    move_out = fix * sa_move + move

Sharding: 8 cores = (batch 2) x (D quarters of 20 planes). Each core gets a
zero-padded input slab of 28 D-planes (4 pad each side; conv halo needs 3)
per tensor and produces 20 output planes. Host shards/pads/gathers.

Per-core pipeline (single HBM read of each input byte):
  - Stream D in chunks of G=4 planes. Load tile layout: partition=(hg32,d4),
    free=(c16, hp3*w96) -> 1152B contiguous DMA lines.
  - Channel max/sum via tensor-tensor trees (max on GpSimd, sum on DVE);
    level 1 casts f32->bf16 so upper levels run in the DVE 2x mode.
  - Pooled planes staged into persistent P tiles [hin_pad128, stat2, dp28,
    wp102] bf16 via SBUF->SBUF reshape DMAs (H/W zero padding lives in P).
  - Conv: 98 accumulated PE matmuls per 4-plane output group; contraction
    over padded H with banded weights lhsT[hin, tap, hout] (kh folded into
    the band; taps enumerate (stat, kd, kw); mean's 1/16 folded in).
  - Sigmoid on ScalarE (PSUM -> SBUF), then reshape to the data layout.
  - Gating in c-quarters: fix chain on DVE, move chain on GpSimd; results
    stream straight back to DRAM.
"""

import sys

import numpy as np

for _p in ("/opt/trn_rl_repo",):
    if _p not in sys.path:
        sys.path.insert(0, _p)

import ml_dtypes  # noqa: E402

B, C, D, H, W = 2, 16, 80, 96, 96
KK = 7
DSLAB = 28          # padded per-core D planes (4 + 20 + 4)
OUTD = 20           # output planes per core
G = 4               # D planes per chunk
NCHUNK = DSLAB // G  # 7
NOC = OUTD // G      # 5
HG, HPW = 32, 3      # h = hg*3 + hp
WPAD = 102
TAPS = 2 * KK * KK   # 98
NCORES = 8
QC = 4               # c-planes per elementwise quarter

_prog_cache: dict = {}


def _build_banded(w: np.ndarray, mean_scale: float) -> np.ndarray:
    """w: [1,2,7,7,7] f32 -> lhsT [hin_pad 128, tap 98, hout 96] bf16.

    out[o,h,w'] = sum_taps lhsT[hq, tap, h] * P[hq, stat, dp=o+1+kd, wp=w'+kw]
    with P[hq=h_in+3, :, :, wp=w_in+3] = pooled and lhsT[h+kh, (c,kd,kw), h]
    = w[c,kd,kh,kw].
    """
    A = np.zeros((128, TAPS, 96), np.float32)
    hh = np.arange(96)
    for c in range(2):
        scale = 1.0 if c == 0 else mean_scale
        for kd in range(KK):
            for kw in range(KK):
                tap = (c * KK + kd) * KK + kw
                for kh in range(KK):
                    A[hh + kh, tap, hh] = w[0, c, kd, kh, kw] * scale
    return A.astype(ml_dtypes.bfloat16)


def _build_program():
    import concourse.bass as bass
    import concourse.bacc as bacc
    import concourse.tile as tile
    from concourse import mybir
    from contextlib import ExitStack

    f32 = mybir.dt.float32
    bf16 = mybir.dt.bfloat16

    nc = bacc.Bacc("TRN2")
    fxs = nc.dram_tensor("fxs", [C, DSLAB, H, W], f32, kind="ExternalInput")
    mvs = nc.dram_tensor("mvs", [C, DSLAB, H, W], f32, kind="ExternalInput")
    wgf = nc.dram_tensor("wgf", [128, TAPS, 96], bf16, kind="ExternalInput")
    wgm = nc.dram_tensor("wgm", [128, TAPS, 96], bf16, kind="ExternalInput")
    fo = nc.dram_tensor("fo", [C, OUTD, H, W], f32, kind="ExternalOutput")
    mo = nc.dram_tensor("mo", [C, OUTD, H, W], f32, kind="ExternalOutput")

    with tile.TileContext(nc) as tc, ExitStack() as ctx:
        singles = ctx.enter_context(tc.tile_pool(name="singles", bufs=1))
        lpf = ctx.enter_context(tc.tile_pool(name="lpf", bufs=3))
        lpm = ctx.enter_context(tc.tile_pool(name="lpm", bufs=3))
        trpool = ctx.enter_context(tc.tile_pool(name="tr", bufs=2))
        tpool = ctx.enter_context(tc.tile_pool(name="tmp", bufs=4))
        pstage = ctx.enter_context(tc.tile_pool(name="pstage", bufs=1))
        gpool = ctx.enter_context(tc.tile_pool(name="gate", bufs=2))
        gtpool = ctx.enter_context(tc.tile_pool(name="gateT", bufs=2))
        psum = ctx.enter_context(tc.tile_pool(name="psum", bufs=4, space="PSUM"))

        WGF = singles.tile([128, TAPS, 96], bf16)
        WGM = singles.tile([128, TAPS, 96], bf16)
        nc.sync.dma_start(out=WGF[:], in_=wgf[:])
        nc.sync.dma_start(out=WGM[:], in_=wgm[:])

        # Persistent pooled tensors [hin_pad, stat, dp, wp]
        PF = singles.tile([128, 2, DSLAB, WPAD], bf16)
        PM = singles.tile([128, 2, DSLAB, WPAD], bf16)
        nc.vector.memset(PF[:], 0.0)
        nc.vector.memset(PM[:], 0.0)

        ltiles: dict = {}

        def load_and_pool(ic: int):
            i0 = G * ic
            for name, dram, lpool, P in (("f", fxs, lpf, PF), ("m", mvs, lpm, PM)):
                # partition order p = d*32 + hg: (d hg) strides chain, so the
                # whole chunk loads as ONE 3-dim DMA (2.36 MB, 1152B lines)
                L = lpool.tile([128, C, HPW * W], f32, tag="L" + name)
                src = dram[:, i0:i0 + G, :, :].rearrange(
                    "c d (hg hp) w -> (d hg) c (hp w)", hg=HG, hp=HPW
                )
                nc.sync.dma_start(out=L[:], in_=src)
                ltiles[(name, ic)] = L

                # channel-reduction trees -> PS [128, stat2, 288] bf16
                PS = pstage.tile([128, 2, HPW * W], bf16, tag="PS")
                TR = trpool.tile([128, 2, C // 2, HPW * W], bf16, tag="TR")
                TRmax, TRsum = TR[:, 0], TR[:, 1]
                # level 1: 16 -> 8 (f32 in, bf16 out)
                nc.vector.tensor_max(TRmax[:, :, :], L[:, 0:8, :], L[:, 8:16, :])
                nc.gpsimd.tensor_add(TRsum[:, :, :], L[:, 0:8, :], L[:, 8:16, :])
                # level 2: 8 -> 4
                nc.vector.tensor_max(TRmax[:, 0:4, :], TRmax[:, 0:4, :], TRmax[:, 4:8, :])
                nc.gpsimd.tensor_add(TRsum[:, 0:4, :], TRsum[:, 0:4, :], TRsum[:, 4:8, :])
                # level 3: 4 -> 2
                nc.vector.tensor_max(TRmax[:, 0:2, :], TRmax[:, 0:2, :], TRmax[:, 2:4, :])
                nc.gpsimd.tensor_add(TRsum[:, 0:2, :], TRsum[:, 0:2, :], TRsum[:, 2:4, :])
                # level 4: 2 -> 1
                nc.vector.tensor_max(PS[:, 0, :], TRmax[:, 0, :], TRmax[:, 1, :])
                nc.gpsimd.tensor_add(PS[:, 1, :], TRsum[:, 0, :], TRsum[:, 1, :])

                # reshape into P: per (d, hp): src partitions d*32..d*32+31
                # (hg), free (stat, w); dst partitions 3+hp+3*hg (step 3),
                # free (stat, wp). Both sides enumerate (hg, stat, w).
                PSv = PS[:].rearrange("p s (hp w) -> p s hp w", hp=HPW)
                for d in range(G):
                    for hp in range(HPW):
                        dst = P[3 + hp:3 + hp + 94:3, :, i0 + d, 3:3 + W]
                        nc.scalar.dma_start(
                            out=dst,
                            in_=PSv[d * HG:(d + 1) * HG, :, hp, :],
                        )

        def conv_group(oc: int):
            o0 = G * oc
            gates = {}
            for name, P, WG in (("f", PF, WGF), ("m", PM, WGM)):
                acc = psum.tile([96, G, 96], mybir.dt.float32, tag="acc")
                for c in range(2):
                    for kd in range(KK):
                        for kw in range(KK):
                            tap = (c * KK + kd) * KK + kw
                            nc.tensor.matmul(
                                acc[:],
                                WG[:, tap, :],
                                P[:, c, o0 + 1 + kd:o0 + 1 + kd + G, kw:kw + 96],
                                start=(tap == 0),
                                stop=(tap == TAPS - 1),
                            )
                gate = gpool.tile([96, G, 96], mybir.dt.bfloat16, tag="gate")
                nc.scalar.activation(
                    out=gate[:], in_=acc[:],
                    func=mybir.ActivationFunctionType.Sigmoid,
                )
                # [96=h, (d,w)] -> gateT [128=(hg,d), (hp,w)]
                gateT = gtpool.tile([128, HPW, W], mybir.dt.bfloat16, tag="gT")
                for d in range(G):
                    nc.scalar.dma_start(
                        out=gateT[d * HG:(d + 1) * HG, :, :], in_=gate[:, d, :]
                    )
                gates[name] = gateT
            return gates

        def elementwise(oc: int, gates):
            ic = oc + 1
            Lf, Lm = ltiles[("f", ic)], ltiles[("m", ic)]
            gf = (
                gates["f"][:].rearrange("p hp w -> p (hp w)").unsqueeze(1)
                .broadcast_to((128, QC, HPW * W))
            )
            gm = (
                gates["m"][:].rearrange("p hp w -> p (hp w)").unsqueeze(1)
                .broadcast_to((128, QC, HPW * W))
            )
            for q in range(C // QC):
                cs = slice(q * QC, (q + 1) * QC)
                Tf = tpool.tile([128, QC, HPW * W], mybir.dt.float32, tag="T")
                Tm = tpool.tile([128, QC, HPW * W], mybir.dt.float32, tag="T")
                # fix chain on DVE: fo = move*gf + fix
                nc.vector.tensor_mul(Tf[:], Lm[:, cs, :], gf)
                nc.vector.tensor_add(Tf[:], Tf[:], Lf[:, cs, :])
                # move chain on GpSimd: mo = fix*gm + move
                nc.gpsimd.tensor_mul(Tm[:], Lf[:, cs, :], gm)
                nc.vector.tensor_add(Tm[:], Tm[:], Lm[:, cs, :])
                for T, dram in ((Tf, fo), (Tm, mo)):
                    dst = dram[cs, G * oc:G * oc + G, :, :].rearrange(
                        "c d (hg hp) w -> (d hg) c (hp w)", hg=HG, hp=HPW
                    )
                    nc.scalar.dma_start(out=dst, in_=T[:])

        # software pipeline
        load_and_pool(0)
        load_and_pool(1)
        load_and_pool(2)
        for oc in range(NOC):
            if oc + 3 < NCHUNK:
                load_and_pool(oc + 3)
            gates = conv_group(oc)
            elementwise(oc, gates)

    nc.compile()
    return nc


def _get_program():
    if "nc" not in _prog_cache:
        _prog_cache["nc"] = _build_program()
    return _prog_cache["nc"]


def _shard(fix, move, Af, Am):
    in_maps = []
    for core in range(NCORES):
        b, dq = core // 4, core % 4
        lo = 20 * dq - 4
        s0, s1 = max(lo, 0), min(lo + DSLAB, D)
        slab_f = np.zeros((C, DSLAB, H, W), np.float32)
        slab_m = np.zeros((C, DSLAB, H, W), np.float32)
        slab_f[:, s0 - lo:s1 - lo] = fix[b, :, s0:s1]
        slab_m[:, s0 - lo:s1 - lo] = move[b, :, s0:s1]
        in_maps.append({"fxs": slab_f, "mvs": slab_m, "wgf": Af, "wgm": Am})
    return in_maps


def kernel(fix, move, w_f2m, w_m2f, __trace=False):
    fix = np.ascontiguousarray(np.asarray(fix), dtype=np.float32)
    move = np.ascontiguousarray(np.asarray(move), dtype=np.float32)
    Af = _build_banded(np.asarray(w_f2m, dtype=np.float32), 1.0 / C)
    Am = _build_banded(np.asarray(w_m2f, dtype=np.float32), 1.0 / C)

    nc = _get_program()
    in_maps = _shard(fix, move, Af, Am)

    from concourse.bass_utils import run_bass_kernel_spmd

    res = run_bass_kernel_spmd(
        nc, in_maps, core_ids=list(range(NCORES)), trace=__trace
    )
    _prog_cache["last_results"] = res

    fix_out = np.empty((B, C, D, H, W), np.float32)
    move_out = np.empty((B, C, D, H, W), np.float32)
    for core in range(NCORES):
        b, dq = core // 4, core % 4
        fix_out[b, :, 20 * dq:20 * dq + 20] = res.results[core]["fo"]
        move_out[b, :, 20 * dq:20 * dq + 20] = res.results[core]["mo"]
    return fix_out, move_out



# revision 10
# speedup vs baseline: 1.8888x; 1.8888x over previous
"""Trainium2 Bass kernel for nn_CSABlock (dual spatial-attention gating).

Reference computation:
    sa_x  = sigmoid(conv3d(concat[max_c(x), mean_c(x)], w, k=7, pad=3))
    fix_out  = move * sa_fix + fix
    move_out = fix * sa_move + move

Sharding: 8 cores = (batch 2) x (D quarters of 20 planes). Each core gets a
zero-padded bf16 input slab of 28 D-planes (4 pad each side; conv halo needs
3) per tensor and produces 20 output planes in bf16. Host casts/shards/pads/
gathers (rel-err budget 2e-2 >> bf16 quantization ~6e-3).

Per-core pipeline (single HBM read of each input byte):
  - Stream D in chunks of G=4 planes into a combined tile L[(d4,hg32),
    t2, c16, hp3*w96] bf16 (576B contiguous lines, one DMA per tensor).
  - Channel max/sum via tensor-tensor trees over both tensors at once
    (max on DVE, sum split DVE/GpSimd); final level casts to fp8e4.
  - Pooled stats round-trip through a DRAM bounce tile to reach the conv
    layout P[hin_pad128, stat2, dp28, wp102] fp8 (2 big DMAs per chunk
    instead of a 12-DMA partition-scatter).
  - Conv: fp8 DoubleRow matmuls; the (stat, h_in) pair packs into the
    256-deep contraction, so taps = (kd, kw) = 49 per 4-plane group with
    kh folded into the banded lhsT [hin, tap, stat, hout] (mean's 1/16
    folded into the avg-channel weights).
  - Sigmoid on ScalarE (PSUM -> SBUF bf16), gate round-trips through DRAM
    to the data layout, broadcast over c.
  - Gating: full-C bf16 mul+add on DVE (2x mode), one store per (group,
    tensor) back to DRAM.
"""

import sys

import numpy as np

for _p in ("/opt/trn_rl_repo",):
    if _p not in sys.path:
        sys.path.insert(0, _p)

import ml_dtypes  # noqa: E402

B, C, D, H, W = 2, 16, 80, 96, 96
KK = 7
DSLAB = 28          # padded per-core D planes (4 + 20 + 4)
OUTD = 20           # output planes per core
G = 4               # D planes per chunk / conv group / elementwise group
NCHUNK = DSLAB // G  # 7
NG = OUTD // G       # 5
HG, HPW = 32, 3      # h = hg*3 + hp
WPAD = 102
NCORES = 8

CONV_FP8 = True     # fp8e4 DoubleRow conv (49 taps) vs bf16 (98 taps)

_prog_cache: dict = {}

_bf16 = ml_dtypes.bfloat16
_f8 = ml_dtypes.float8_e4m3


def _build_banded_fp8(w: np.ndarray, mean_scale: float) -> np.ndarray:
    """w: [1,2,7,7,7] f32 -> lhsT [hin_pad 128, tap 49, stat 2, hout 96] fp8.

    out[h,*] += lhsT[h+kh, (kd,kw), s, h] * P[h+kh, s, o+1+kd, w+kw]
    """
    A = np.zeros((128, KK * KK, 2, 96), np.float32)
    hh = np.arange(96)
    for s in range(2):
        scale = 1.0 if s == 0 else mean_scale
        for kd in range(KK):
            for kw in range(KK):
                tap = kd * KK + kw
                for kh in range(KK):
                    A[hh + kh, tap, s, hh] = w[0, s, kd, kh, kw] * scale
    return A.astype(_f8)


def _build_banded_bf16(w: np.ndarray, mean_scale: float) -> np.ndarray:
    """w: [1,2,7,7,7] f32 -> lhsT [hin_pad 128, tap 98, hout 96] bf16."""
    A = np.zeros((128, 2 * KK * KK, 96), np.float32)
    hh = np.arange(96)
    for s in range(2):
        scale = 1.0 if s == 0 else mean_scale
        for kd in range(KK):
            for kw in range(KK):
                tap = (s * KK + kd) * KK + kw
                for kh in range(KK):
                    A[hh + kh, tap, hh] = w[0, s, kd, kh, kw] * scale
    return A.astype(_bf16)


def _build_program():
    import concourse.bass as bass  # noqa: F401
    import concourse.bacc as bacc
    import concourse.tile as tile
    from concourse import mybir
    from contextlib import ExitStack

    f32 = mybir.dt.float32
    bf16 = mybir.dt.bfloat16
    f16 = mybir.dt.float16
    f8 = mybir.dt.float8e4
    pdt = f8 if CONV_FP8 else bf16
    TAPS = KK * KK if CONV_FP8 else 2 * KK * KK

    nc = bacc.Bacc("TRN2")
    fxs = nc.dram_tensor("fxs", [C, DSLAB, H, W], bf16, kind="ExternalInput")
    mvs = nc.dram_tensor("mvs", [C, DSLAB, H, W], bf16, kind="ExternalInput")
    if CONV_FP8:
        wgf = nc.dram_tensor("wgf", [128, TAPS, 2, 96], f8, kind="ExternalInput")
        wgm = nc.dram_tensor("wgm", [128, TAPS, 2, 96], f8, kind="ExternalInput")
    else:
        wgf = nc.dram_tensor("wgf", [128, TAPS, 96], bf16, kind="ExternalInput")
        wgm = nc.dram_tensor("wgm", [128, TAPS, 96], bf16, kind="ExternalInput")
    fo = nc.dram_tensor("fo", [C, OUTD, H, W], bf16, kind="ExternalOutput")
    mo = nc.dram_tensor("mo", [C, OUTD, H, W], bf16, kind="ExternalOutput")

    with tile.TileContext(nc) as tc, ExitStack() as ctx:
        singles = ctx.enter_context(tc.tile_pool(name="singles", bufs=1))
        lp = ctx.enter_context(tc.tile_pool(name="lp", bufs=4))
        trpool = ctx.enter_context(tc.tile_pool(name="tr", bufs=2))
        pspool = ctx.enter_context(tc.tile_pool(name="ps", bufs=2))
        tpool = ctx.enter_context(tc.tile_pool(name="tmp", bufs=3))
        gpool = ctx.enter_context(tc.tile_pool(name="gate", bufs=2))
        gtpool = ctx.enter_context(tc.tile_pool(name="gateT", bufs=3))
        psum = ctx.enter_context(tc.tile_pool(name="psum", bufs=4, space="PSUM"))
        dram = ctx.enter_context(tc.tile_pool(name="dram", bufs=1, space="DRAM"))

        WGF = singles.tile(list(wgf.shape), pdt)
        WGM = singles.tile(list(wgm.shape), pdt)
        nc.scalar.dma_start(out=WGF[:], in_=wgf[:])
        nc.scalar.dma_start(out=WGM[:], in_=wgm[:])

        # Persistent pooled tensors [hin_pad, stat, dp*wp] ((d,w) flattened so
        # conv rhs slices are single contiguous runs; the 6 pad columns per
        # plane become ignored output columns)
        PF = singles.tile([128, 2, DSLAB * WPAD], pdt)
        PM = singles.tile([128, 2, DSLAB * WPAD], pdt)
        nc.gpsimd.memset(PF[:], 0.0)
        nc.gpsimd.memset(PM[:], 0.0)

        # DRAM bounce tiles
        pooled_d = [
            dram.tile([2, 2, G, H, W], pdt, name=f"pooled_d{i}")
            for i in range(NCHUNK)
        ]
        gates_d = {
            (t, g): dram.tile([G, H, W], bf16, name=f"gates_d{t}_{g}")
            for t in range(2) for g in range(NG)
        }

        ltiles: dict = {}

        def load_and_pool(ic: int):
            i0 = G * ic
            # Combined tile: partition (d4, hg32); free (t2, c16, hp3*w96)
            L = lp.tile([128, 2, C, HPW * W], bf16, tag="L")
            for t, dram_in in ((0, fxs), (1, mvs)):
                src = dram_in[:, i0:i0 + G, :, :].rearrange(
                    "c d (hg hp) w -> (d hg) c (hp w)", hg=HG, hp=HPW
                )
                nc.sync.dma_start(out=L[:, t], in_=src)
            ltiles[ic] = L

            # channel-reduction trees over both tensors at once (fp16: exact
            # for bf16 inputs, keeps DVE 2x mode, avoids bf16 sum noise)
            TR = trpool.tile([128, 2, 2, C // 2, HPW * W], f16, tag="TR")
            PS = pspool.tile([128, 2, 2, HPW * W], pdt, tag="PS")
            TRmax, TRsum = TR[:, :, 0], TR[:, :, 1]
            # level 1: 16 -> 8
            nc.vector.tensor_max(TRmax[:, :, :, :], L[:, :, 0:8, :], L[:, :, 8:16, :])
            nc.vector.tensor_add(TRsum[:, :, :, :], L[:, :, 0:8, :], L[:, :, 8:16, :])
            # levels 2-4: max on DVE, sum on GpSimd
            nc.vector.tensor_max(TRmax[:, :, 0:4], TRmax[:, :, 0:4], TRmax[:, :, 4:8])
            nc.gpsimd.tensor_add(TRsum[:, :, 0:4], TRsum[:, :, 0:4], TRsum[:, :, 4:8])
            nc.vector.tensor_max(TRmax[:, :, 0:2], TRmax[:, :, 0:2], TRmax[:, :, 2:4])
            nc.gpsimd.tensor_add(TRsum[:, :, 0:2], TRsum[:, :, 0:2], TRsum[:, :, 2:4])
            nc.vector.tensor_max(PS[:, :, 0], TRmax[:, :, 0], TRmax[:, :, 1])
            nc.gpsimd.tensor_add(TRsum[:, :, 0], TRsum[:, :, 0], TRsum[:, :, 1])
            # mean = sum/16 applied here on ScalarE: folding 1/16 into fp8
            # weights would push them into e4m3 subnormals
            nc.scalar.mul(PS[:, :, 1], TRsum[:, :, 0], 1.0 / C)

            # bounce: PS [(d hg), t, s, (hp w)] -> DRAM [t, s, d, h, w]
            nc.scalar.dma_start(
                out=pooled_d[ic][:].rearrange(
                    "t s d (hg hp) w -> (d hg) t s (hp w)", hg=HG, hp=HPW
                ),
                in_=PS[:],
            )
            # reload into conv layout per tensor/stat (3-dim DMA APs)
            for t, P in ((0, PF), (1, PM)):
                Pv = P[3:99, :, :].rearrange("p s (d w) -> p s d w", w=WPAD)
                for s in range(2):
                    nc.sync.dma_start(
                        out=Pv[:, s, i0:i0 + G, 3:3 + W],
                        in_=pooled_d[ic][t, s].rearrange("d h w -> h d w"),
                    )

        NFREE = G * WPAD - (WPAD - W)  # 402: contiguous (d,w) run per tap

        def conv_group(t: int, g: int):
            P = (PF, PM)[t]
            WG = (WGF, WGM)[t]
            o0 = G * g
            acc = psum.tile([96, NFREE], f32, tag="acc")
            if CONV_FP8:
                for kd in range(KK):
                    for kw in range(KK):
                        tap = kd * KK + kw
                        off = (o0 + 1 + kd) * WPAD + kw
                        nc.tensor.matmul(
                            acc[:],
                            WG[:, tap],
                            P[:, :, off:off + NFREE],
                            start=(tap == 0),
                            stop=(tap == TAPS - 1),
                            perf_mode=mybir.MatmulPerfMode.DoubleRow,
                        )
            else:
                for s in range(2):
                    for kd in range(KK):
                        for kw in range(KK):
                            tap = (s * KK + kd) * KK + kw
                            off = (o0 + 1 + kd) * WPAD + kw
                            nc.tensor.matmul(
                                acc[:],
                                WG[:, tap],
                                P[:, s, off:off + NFREE],
                                start=(tap == 0),
                                stop=(tap == TAPS - 1),
                            )
            gate = gpool.tile([96, G * WPAD], bf16, tag="gate")
            nc.scalar.activation(
                out=gate[:, 0:NFREE], in_=acc[:],
                func=mybir.ActivationFunctionType.Sigmoid,
            )
            gv = gate[:].rearrange("p (d w) -> p d w", w=WPAD)
            nc.scalar.dma_start(
                out=gates_d[(t, g)][:].rearrange("d h w -> h d w"),
                in_=gv[:, :, 0:W],
            )

        def elementwise(g: int):
            L = ltiles[g + 1]
            Ts = []
            for t in range(2):
                gateT = gtpool.tile([128, HPW * W], bf16, tag="gT")
                nc.sync.dma_start(
                    out=gateT[:],
                    in_=gates_d[(t, g)][:].rearrange(
                        "d (hg hp) w -> (d hg) (hp w)", hg=HG, hp=HPW
                    ),
                )
                gb = gateT[:].unsqueeze(1).broadcast_to((128, C, HPW * W))
                T = tpool.tile([128, C, HPW * W], bf16, tag="T")
                # t=0: fix_out = move*gf + fix ; t=1: move_out = fix*gm + move
                nc.vector.tensor_mul(T[:], L[:, 1 - t], gb)
                nc.vector.tensor_add(T[:], T[:], L[:, t])
                Ts.append(T)
            for t, dram_out in ((0, fo), (1, mo)):
                dst = dram_out[:, G * g:G * g + G, :, :].rearrange(
                    "c d (hg hp) w -> (d hg) c (hp w)", hg=HG, hp=HPW
                )
                nc.scalar.dma_start(out=dst, in_=Ts[t][:])

        # software pipeline: conv group g needs pooled chunks <= g+2;
        # elementwise g needs raw chunk g+1 and group-g gates.
        load_and_pool(0)
        load_and_pool(1)
        load_and_pool(2)
        for g in range(NG):
            if g + 3 < NCHUNK:
                load_and_pool(g + 3)
            conv_group(0, g)
            conv_group(1, g)
            elementwise(g)

    nc.compile()
    return nc


def _get_program():
    if "nc" not in _prog_cache:
        _prog_cache["nc"] = _build_program()
    return _prog_cache["nc"]


def _shard(fix, move, Af, Am):
    in_maps = []
    for core in range(NCORES):
        b, dq = core // 4, core % 4
        lo = 20 * dq - 4
        s0, s1 = max(lo, 0), min(lo + DSLAB, D)
        slab_f = np.zeros((C, DSLAB, H, W), _bf16)
        slab_m = np.zeros((C, DSLAB, H, W), _bf16)
        slab_f[:, s0 - lo:s1 - lo] = fix[b, :, s0:s1].astype(_bf16)
        slab_m[:, s0 - lo:s1 - lo] = move[b, :, s0:s1].astype(_bf16)
        in_maps.append({"fxs": slab_f, "mvs": slab_m, "wgf": Af, "wgm": Am})
    return in_maps


def _build_weights(w_f2m, w_m2f):
    build = _build_banded_fp8 if CONV_FP8 else _build_banded_bf16
    Af = build(np.asarray(w_f2m, dtype=np.float32), 1.0)
    Am = build(np.asarray(w_m2f, dtype=np.float32), 1.0)
    return Af, Am


def kernel(fix, move, w_f2m, w_m2f, __trace=False):
    fix = np.ascontiguousarray(np.asarray(fix), dtype=np.float32)
    move = np.ascontiguousarray(np.asarray(move), dtype=np.float32)
    Af, Am = _build_weights(w_f2m, w_m2f)

    nc = _get_program()
    in_maps = _shard(fix, move, Af, Am)

    from concourse.bass_utils import run_bass_kernel_spmd

    res = run_bass_kernel_spmd(
        nc, in_maps, core_ids=list(range(NCORES)), trace=__trace
    )
    _prog_cache["last_results"] = res

    fix_out = np.empty((B, C, D, H, W), np.float32)
    move_out = np.empty((B, C, D, H, W), np.float32)
    for core in range(NCORES):
        b, dq = core // 4, core % 4
        fix_out[b, :, 20 * dq:20 * dq + 20] = res.results[core]["fo"].astype(np.float32)
        move_out[b, :, 20 * dq:20 * dq + 20] = res.results[core]["mo"].astype(np.float32)
    return fix_out, move_out


# revision 11
# speedup vs baseline: 2.1147x; 1.1196x over previous
"""Trainium2 Bass kernel for nn_CSABlock (dual spatial-attention gating).

Reference computation:
    sa_x  = sigmoid(conv3d(concat[max_c(x), mean_c(x)], w, k=7, pad=3))
    fix_out  = move * sa_fix + fix
    move_out = fix * sa_move + move

Sharding: 8 cores = (batch 2) x (D quarters of 20 planes). Each core gets a
zero-padded bf16 input slab of 28 D-planes (4 pad each side; conv halo needs
3) per tensor and produces 20 output planes in bf16. Host casts/shards/pads/
gathers (rel-err budget 2e-2 >> bf16 quantization ~6e-3).

Per-core pipeline (single HBM read of each input byte):
  - Stream D in chunks of G=4 planes into a combined tile L[(d4,hg32),
    t2, c16, hp3*w96] bf16 (576B contiguous lines, one DMA per tensor).
  - Channel max/sum via tensor-tensor trees over both tensors at once
    (max on DVE, sum split DVE/GpSimd); final level casts to fp8e4.
  - Pooled stats round-trip through a DRAM bounce tile to reach the conv
    layout P[hin_pad128, stat2, dp28, wp102] fp8 (2 big DMAs per chunk
    instead of a 12-DMA partition-scatter).
  - Conv: fp8 DoubleRow matmuls; the (stat, h_in) pair packs into the
    256-deep contraction, so taps = (kd, kw) = 49 per 4-plane group with
    kh folded into the banded lhsT [hin, tap, stat, hout] (mean's 1/16
    folded into the avg-channel weights).
  - Sigmoid on ScalarE (PSUM -> SBUF bf16), gate round-trips through DRAM
    to the data layout, broadcast over c.
  - Gating: full-C bf16 mul+add on DVE (2x mode), one store per (group,
    tensor) back to DRAM.
"""

import sys

import numpy as np

for _p in ("/opt/trn_rl_repo",):
    if _p not in sys.path:
        sys.path.insert(0, _p)

import ml_dtypes  # noqa: E402

B, C, D, H, W = 2, 16, 80, 96, 96
KK = 7
DSLAB = 28          # padded per-core D planes (4 + 20 + 4)
OUTD = 20           # output planes per core
G = 4               # D planes per chunk / conv group / elementwise group
NCHUNK = DSLAB // G  # 7
NG = OUTD // G       # 5
HG, HPW = 32, 3      # h = hg*3 + hp
WPAD = 102
NCORES = 8

CONV_FP8 = True     # fp8e4 DoubleRow conv (49 taps) vs bf16 (98 taps)

_prog_cache: dict = {}

_bf16 = ml_dtypes.bfloat16
_f8 = ml_dtypes.float8_e4m3


def _build_banded_fp8(w: np.ndarray, mean_scale: float) -> np.ndarray:
    """w: [1,2,7,7,7] f32 -> lhsT [hin_pad 128, tap 49, stat 2, hout 96] fp8.

    out[h,*] += lhsT[h+kh, (kd,kw), s, h] * P[h+kh, s, o+1+kd, w+kw]
    """
    A = np.zeros((128, KK * KK, 2, 96), np.float32)
    hh = np.arange(96)
    for s in range(2):
        scale = 1.0 if s == 0 else mean_scale
        for kd in range(KK):
            for kw in range(KK):
                tap = kd * KK + kw
                for kh in range(KK):
                    A[hh + kh, tap, s, hh] = w[0, s, kd, kh, kw] * scale
    return A.astype(_f8)


def _build_banded_bf16(w: np.ndarray, mean_scale: float) -> np.ndarray:
    """w: [1,2,7,7,7] f32 -> lhsT [hin_pad 128, tap 98, hout 96] bf16."""
    A = np.zeros((128, 2 * KK * KK, 96), np.float32)
    hh = np.arange(96)
    for s in range(2):
        scale = 1.0 if s == 0 else mean_scale
        for kd in range(KK):
            for kw in range(KK):
                tap = (s * KK + kd) * KK + kw
                for kh in range(KK):
                    A[hh + kh, tap, hh] = w[0, s, kd, kh, kw] * scale
    return A.astype(_bf16)


def _build_program():
    import concourse.bass as bass  # noqa: F401
    import concourse.bacc as bacc
    import concourse.tile as tile
    from concourse import mybir
    from contextlib import ExitStack

    f32 = mybir.dt.float32
    bf16 = mybir.dt.bfloat16
    f16 = mybir.dt.float16
    f8 = mybir.dt.float8e4
    pdt = f8 if CONV_FP8 else bf16
    TAPS = KK * KK if CONV_FP8 else 2 * KK * KK

    nc = bacc.Bacc("TRN2")
    fxs = nc.dram_tensor("fxs", [C, DSLAB, H, W], bf16, kind="ExternalInput")
    mvs = nc.dram_tensor("mvs", [C, DSLAB, H, W], bf16, kind="ExternalInput")
    if CONV_FP8:
        wgf = nc.dram_tensor("wgf", [128, TAPS, 2, 96], f8, kind="ExternalInput")
        wgm = nc.dram_tensor("wgm", [128, TAPS, 2, 96], f8, kind="ExternalInput")
    else:
        wgf = nc.dram_tensor("wgf", [128, TAPS, 96], bf16, kind="ExternalInput")
        wgm = nc.dram_tensor("wgm", [128, TAPS, 96], bf16, kind="ExternalInput")
    fo = nc.dram_tensor("fo", [C, OUTD, H, W], bf16, kind="ExternalOutput")
    mo = nc.dram_tensor("mo", [C, OUTD, H, W], bf16, kind="ExternalOutput")

    with tile.TileContext(nc) as tc, ExitStack() as ctx:
        singles = ctx.enter_context(tc.tile_pool(name="singles", bufs=1))
        lp = ctx.enter_context(tc.tile_pool(name="lp", bufs=4))
        trpool = ctx.enter_context(tc.tile_pool(name="tr", bufs=2))
        pspool = ctx.enter_context(tc.tile_pool(name="ps", bufs=2))
        tpool = ctx.enter_context(tc.tile_pool(name="tmp", bufs=3))
        gpool = ctx.enter_context(tc.tile_pool(name="gate", bufs=2))
        gtpool = ctx.enter_context(tc.tile_pool(name="gateT", bufs=3))
        psum = ctx.enter_context(tc.tile_pool(name="psum", bufs=4, space="PSUM"))
        dram = ctx.enter_context(tc.tile_pool(name="dram", bufs=1, space="DRAM"))

        WGF = singles.tile(list(wgf.shape), pdt)
        WGM = singles.tile(list(wgm.shape), pdt)
        nc.scalar.dma_start(out=WGF[:], in_=wgf[:])
        nc.scalar.dma_start(out=WGM[:], in_=wgm[:])

        # Persistent pooled tensors [hin_pad, stat, dp*wp] ((d,w) flattened so
        # conv rhs slices are single contiguous runs; the 6 pad columns per
        # plane become ignored output columns)
        PF = singles.tile([128, 2, DSLAB * WPAD], pdt)
        PM = singles.tile([128, 2, DSLAB * WPAD], pdt)
        nc.gpsimd.memset(PF[:], 0.0)
        nc.gpsimd.memset(PM[:], 0.0)

        # DRAM bounce tiles
        pooled_d = [
            dram.tile([2, 2, G, H, W], pdt, name=f"pooled_d{i}")
            for i in range(NCHUNK)
        ]
        gates_d = {
            (t, g): dram.tile([G, H, W], bf16, name=f"gates_d{t}_{g}")
            for t in range(2) for g in range(NG)
        }

        ltiles: dict = {}

        def load_and_pool(ic: int):
            i0 = G * ic
            # Combined tile: partition (d4, hg32); free (t2, c16, hp3*w96)
            L = lp.tile([128, 2, C, HPW * W], bf16, tag="L")
            for t, dram_in in ((0, fxs), (1, mvs)):
                src = dram_in[:, i0:i0 + G, :, :].rearrange(
                    "c d (hg hp) w -> (d hg) c (hp w)", hg=HG, hp=HPW
                )
                nc.sync.dma_start(out=L[:, t], in_=src)
            ltiles[ic] = L

            # channel-reduction trees over both tensors at once (fp16: exact
            # for bf16 inputs, keeps DVE 2x mode, avoids bf16 sum noise)
            TR = trpool.tile([128, 2, 2, C // 2, HPW * W], f16, tag="TR")
            PS = pspool.tile([128, 2, 2, HPW * W], pdt, tag="PS")
            TRmax, TRsum = TR[:, :, 0], TR[:, :, 1]
            # level 1: 16 -> 8
            nc.vector.tensor_max(TRmax[:, :, :, :], L[:, :, 0:8, :], L[:, :, 8:16, :])
            nc.vector.tensor_add(TRsum[:, :, :, :], L[:, :, 0:8, :], L[:, :, 8:16, :])
            # levels 2-4 all on DVE: GpSimd tensor ops hold the shared SBUF
            # port for multi-us and stall every concurrent DVE perf-mode op
            nc.vector.tensor_max(TRmax[:, :, 0:4], TRmax[:, :, 0:4], TRmax[:, :, 4:8])
            nc.vector.tensor_add(TRsum[:, :, 0:4], TRsum[:, :, 0:4], TRsum[:, :, 4:8])
            nc.vector.tensor_max(TRmax[:, :, 0:2], TRmax[:, :, 0:2], TRmax[:, :, 2:4])
            nc.vector.tensor_add(TRsum[:, :, 0:2], TRsum[:, :, 0:2], TRsum[:, :, 2:4])
            nc.vector.tensor_max(PS[:, :, 0], TRmax[:, :, 0], TRmax[:, :, 1])
            nc.vector.tensor_add(TRsum[:, :, 0], TRsum[:, :, 0], TRsum[:, :, 1])
            # mean = sum/16 applied here on ScalarE: folding 1/16 into fp8
            # weights would push them into e4m3 subnormals
            nc.scalar.mul(PS[:, :, 1], TRsum[:, :, 0], 1.0 / C)

            # bounce: PS [(d hg), t, s, (hp w)] -> DRAM [t, s, d, h, w]
            nc.scalar.dma_start(
                out=pooled_d[ic][:].rearrange(
                    "t s d (hg hp) w -> (d hg) t s (hp w)", hg=HG, hp=HPW
                ),
                in_=PS[:],
            )
            # reload into conv layout per tensor/stat (3-dim DMA APs)
            for t, P in ((0, PF), (1, PM)):
                Pv = P[3:99, :, :].rearrange("p s (d w) -> p s d w", w=WPAD)
                for s in range(2):
                    nc.sync.dma_start(
                        out=Pv[:, s, i0:i0 + G, 3:3 + W],
                        in_=pooled_d[ic][t, s].rearrange("d h w -> h d w"),
                    )

        NFREE = G * WPAD - (WPAD - W)  # 402: contiguous (d,w) run per tap

        def conv_group(t: int, g: int):
            P = (PF, PM)[t]
            WG = (WGF, WGM)[t]
            o0 = G * g
            acc = psum.tile([96, NFREE], f32, tag="acc")
            if CONV_FP8:
                for kd in range(KK):
                    for kw in range(KK):
                        tap = kd * KK + kw
                        off = (o0 + 1 + kd) * WPAD + kw
                        nc.tensor.matmul(
                            acc[:],
                            WG[:, tap],
                            P[:, :, off:off + NFREE],
                            start=(tap == 0),
                            stop=(tap == TAPS - 1),
                            perf_mode=mybir.MatmulPerfMode.DoubleRow,
                        )
            else:
                for s in range(2):
                    for kd in range(KK):
                        for kw in range(KK):
                            tap = (s * KK + kd) * KK + kw
                            off = (o0 + 1 + kd) * WPAD + kw
                            nc.tensor.matmul(
                                acc[:],
                                WG[:, tap],
                                P[:, s, off:off + NFREE],
                                start=(tap == 0),
                                stop=(tap == TAPS - 1),
                            )
            gate = gpool.tile([96, G * WPAD], bf16, tag="gate")
            nc.scalar.activation(
                out=gate[:, 0:NFREE], in_=acc[:],
                func=mybir.ActivationFunctionType.Sigmoid,
            )
            gv = gate[:].rearrange("p (d w) -> p d w", w=WPAD)
            nc.scalar.dma_start(
                out=gates_d[(t, g)][:].rearrange("d h w -> h d w"),
                in_=gv[:, :, 0:W],
            )

        def elementwise(g: int):
            L = ltiles[g + 1]
            Ts = []
            for t in range(2):
                gateT = gtpool.tile([128, HPW * W], bf16, tag="gT")
                nc.sync.dma_start(
                    out=gateT[:],
                    in_=gates_d[(t, g)][:].rearrange(
                        "d (hg hp) w -> (d hg) (hp w)", hg=HG, hp=HPW
                    ),
                )
                gb = gateT[:].unsqueeze(1).broadcast_to((128, C, HPW * W))
                T = tpool.tile([128, C, HPW * W], bf16, tag="T")
                # t=0: fix_out = move*gf + fix ; t=1: move_out = fix*gm + move
                nc.vector.tensor_mul(T[:], L[:, 1 - t], gb)
                nc.vector.tensor_add(T[:], T[:], L[:, t])
                Ts.append(T)
            for t, dram_out in ((0, fo), (1, mo)):
                dst = dram_out[:, G * g:G * g + G, :, :].rearrange(
                    "c d (hg hp) w -> (d hg) c (hp w)", hg=HG, hp=HPW
                )
                nc.scalar.dma_start(out=dst, in_=Ts[t][:])

        # software pipeline: conv group g needs pooled chunks <= g+2;
        # elementwise g needs raw chunk g+1 and group-g gates.
        load_and_pool(0)
        load_and_pool(1)
        load_and_pool(2)
        for g in range(NG):
            if g + 3 < NCHUNK:
                load_and_pool(g + 3)
            conv_group(0, g)
            conv_group(1, g)
            elementwise(g)

    nc.compile()
    return nc


def _get_program():
    if "nc" not in _prog_cache:
        _prog_cache["nc"] = _build_program()
    return _prog_cache["nc"]


def _shard(fix, move, Af, Am):
    in_maps = []
    for core in range(NCORES):
        b, dq = core // 4, core % 4
        lo = 20 * dq - 4
        s0, s1 = max(lo, 0), min(lo + DSLAB, D)
        slab_f = np.zeros((C, DSLAB, H, W), _bf16)
        slab_m = np.zeros((C, DSLAB, H, W), _bf16)
        slab_f[:, s0 - lo:s1 - lo] = fix[b, :, s0:s1].astype(_bf16)
        slab_m[:, s0 - lo:s1 - lo] = move[b, :, s0:s1].astype(_bf16)
        in_maps.append({"fxs": slab_f, "mvs": slab_m, "wgf": Af, "wgm": Am})
    return in_maps


def _build_weights(w_f2m, w_m2f):
    build = _build_banded_fp8 if CONV_FP8 else _build_banded_bf16
    Af = build(np.asarray(w_f2m, dtype=np.float32), 1.0)
    Am = build(np.asarray(w_m2f, dtype=np.float32), 1.0)
    return Af, Am


def kernel(fix, move, w_f2m, w_m2f, __trace=False):
    fix = np.ascontiguousarray(np.asarray(fix), dtype=np.float32)
    move = np.ascontiguousarray(np.asarray(move), dtype=np.float32)
    Af, Am = _build_weights(w_f2m, w_m2f)

    nc = _get_program()
    in_maps = _shard(fix, move, Af, Am)

    from concourse.bass_utils import run_bass_kernel_spmd

    res = run_bass_kernel_spmd(
        nc, in_maps, core_ids=list(range(NCORES)), trace=__trace
    )
    _prog_cache["last_results"] = res

    fix_out = np.empty((B, C, D, H, W), np.float32)
    move_out = np.empty((B, C, D, H, W), np.float32)
    for core in range(NCORES):
        b, dq = core // 4, core % 4
        fix_out[b, :, 20 * dq:20 * dq + 20] = res.results[core]["fo"].astype(np.float32)
        move_out[b, :, 20 * dq:20 * dq + 20] = res.results[core]["mo"].astype(np.float32)
    return fix_out, move_out
